# revision 9
# baseline (speedup 1.0000x reference)
"""Trainium2 Bass kernel for DynamicTokenMixing (16-head attention, N=4096, C=1024).

Sharding: head-parallel across 8 NeuronCores, 2 heads per core. Each core
computes q/k/v projections for its 2 heads, full attention for those heads,
and a partial output projection (row-parallel over Wproj); the host sums the
8 partials and adds the bias.

Per-core dataflow (ACT-engine-bound design; exp of 33.5M scores is the floor):
  fp16 inputs (x, weights) -> kT/qT [dims, tokens] fp16, v [tokens, dims] fp16
  ST[j,i] = sum_d k[j,d] q[i,d]      per-head, K=64 row-tiled so the two
                                     heads' matmuls run concurrently (T0/T8)
  ET      = exp(0.5*ST)              bf16 (fp32-like range, no overflow)
  AV^T    = sum_j v_ext[j,:]^T ET[j,:]  (row 64 = softmax denominator)
  outT    = AV^T[0:64] * recip(denom)   recip broadcast via GPSIMD
  out     = sum_h outT_h.T @ Wproj_rows_h  (fp16 partial; host sums 8 cores)
"""

import numpy as np

import concourse.bass as bass
import concourse.mybir as mybir
import concourse.tile as tile
from concourse import bacc
from concourse import library_config
from concourse.bass_utils import run_bass_kernel_spmd

F32 = mybir.dt.float32
F16 = mybir.dt.float16
BF16 = mybir.dt.bfloat16

N = 4096          # tokens
C = 1024          # model dim
D = 64            # head dim
NHEADS = 16
GPD = 2
NCORES = 8
NJ = N // 128     # 32 key tiles
NCT = C // 128    # 8 contraction tiles
STRIP = 512       # query-strip width
NSTRIP = N // STRIP
JB = 2            # key tiles batched per exp instruction
SCORE_SCALE = GPD * GPD * (D ** -0.5)  # 0.5
USE_GPSIMD_BC = True  # broadcast 1/l via GPSIMD; else stage+matmul fallback


def build_nc(repeat=1, hw_loop=False):
    nc = bacc.Bacc("TRN2", target_bir_lowering=False, debug=False,
                   num_devices=NCORES)
    xT = nc.declare_dram_parameter("xT", [C, N], F16, isOutput=False)
    wq = nc.declare_dram_parameter("wq", [C, 128], F16, isOutput=False)
    wk = nc.declare_dram_parameter("wk", [C, 128], F16, isOutput=False)
    wv = nc.declare_dram_parameter("wv", [C, 128], F16, isOutput=False)
    wpa = nc.declare_dram_parameter("wpa", [D, C], F16, isOutput=False)
    wpb = nc.declare_dram_parameter("wpb", [D, C], F16, isOutput=False)
    out = nc.declare_dram_parameter("out", [N, C], F16, isOutput=True)

    xT_r = xT[:].rearrange("(t p) n -> p t n", p=128)    # [128, 8, 4096]
    out_r = out[:].rearrange("(t p) o -> t p o", p=128)  # [32, 128, 1024]

    with tile.TileContext(nc) as tc:
        with (
            nc.allow_low_precision(reason="fp16 matmul inputs by design"),
            tc.tile_pool(name="persist", bufs=1) as persist,
            tc.tile_pool(name="small", bufs=2) as small,
        ):
            wq_sb = persist.tile([128, NCT, 128], F16)
            wk_sb = persist.tile([128, NCT, 128], F16)
            wv_sb = persist.tile([128, NCT, 128], F16)
            wpa_sb = persist.tile([D, C], F16)
            wpb_sb = persist.tile([D, C], F16)
            xt = persist.tile([128, NCT, N], F16)         # whole x^T resident
            kT_s = [persist.tile([128, STRIP], F16, name=f"kT{i}")
                    for i in range(NSTRIP)]
            # v in natural [token, (j, head, d|ones)] layout, one tile so the
            # ones columns are a single memset
            vsb = persist.tile([128, NJ, 2, 65], BF16)

            nc.sync.dma_start(wq_sb[:], wq[:].rearrange("(t p) m -> p t m", p=128))
            nc.sync.dma_start(wk_sb[:], wk[:].rearrange("(t p) m -> p t m", p=128))
            nc.sync.dma_start(wv_sb[:], wv[:].rearrange("(t p) m -> p t m", p=128))
            nc.sync.dma_start(wpa_sb[:], wpa[:])
            nc.sync.dma_start(wpb_sb[:], wpb[:])
            if USE_GPSIMD_BC:
                nc.gpsimd.load_library(library_config.attn)
            nc.gpsimd.memset(vsb[:, :, :, 64:65], 1.0)
            if not USE_GPSIMD_BC:
                ones_t = persist.tile([1, D], F16)
                nc.gpsimd.memset(ones_t[:], 1.0)

            import contextlib
            rep_iter = ([None] if hw_loop and repeat > 1 else range(repeat))
            for _rep in rep_iter:
              with (tc.For_i(0, repeat, 1) if hw_loop and repeat > 1
                    else contextlib.nullcontext()):
                  # ---- Phase 1: k/v projections (q is JIT per-strip later) ----
                  with (
                      tc.tile_pool(name="kv_ps", bufs=2, space="PSUM") as kv_ps,
                  ):
                      for i in range(NSTRIP):
                          sl = bass.ts(i, STRIP)
                          nc.sync.dma_start(xt[:, :, sl], xT_r[:, :, sl])
                          k_ps = kv_ps.tile([128, STRIP], F32, tag="k")
                          for c in range(NCT):
                              nc.tensor.matmul(k_ps[:], wk_sb[:, c, :],
                                               xt[:, c, sl],
                                               start=(c == 0), stop=(c == NCT - 1))
                          nc.vector.tensor_copy(kT_s[i][:], k_ps[:])
                          for t in range(STRIP // 128):
                              j = i * (STRIP // 128) + t
                              tsl = slice(i * STRIP + t * 128,
                                          i * STRIP + (t + 1) * 128)
                              v_ps = kv_ps.tile([128, 2, 64], F32, tag="v")
                              for c in range(NCT):
                                  nc.tensor.matmul(v_ps[:], xt[:, c, tsl],
                                                   wv_sb[:, c, :],
                                                   start=(c == 0),
                                                   stop=(c == NCT - 1))
                              nc.vector.tensor_copy(vsb[:, j, :, 0:64], v_ps[:])

                  # ---- Phase 2: attention + projection, pipelined per strip ----
                  with (
                      tc.tile_pool(name="qT", bufs=2) as qT_pool,
                      tc.tile_pool(name="att_et", bufs=3) as et_pool,
                      tc.tile_pool(name="outT", bufs=2) as outT_pool,
                      tc.tile_pool(name="ob", bufs=3) as ob_pool,
                      tc.tile_pool(name="att_st", bufs=1, space="PSUM") as st_pool,
                      tc.tile_pool(name="att_av", bufs=1, space="PSUM") as av_pool,
                      tc.tile_pool(name="sc_ps", bufs=2, space="PSUM") as sc_pool,
                  ):
                      for i in range(NSTRIP):
                          sl = bass.ts(i, STRIP)
                          # JIT q projection for this strip
                          q_ps = sc_pool.tile([128, STRIP], F32, tag="sc")
                          for c in range(NCT):
                              nc.tensor.matmul(q_ps[:], wq_sb[:, c, :],
                                               xt[:, c, sl],
                                               start=(c == 0), stop=(c == NCT - 1))
                          qT = qT_pool.tile([128, STRIP], F16, tag="qT")
                          nc.vector.tensor_copy(qT[:], q_ps[:])

                          av = {h: av_pool.tile([65, STRIP], F32, tag=f"av{h}",
                                                name=f"av{h}")
                                for h in (0, 1)}
                          heads = ((0, slice(0, 64)), (1, slice(64, 128)))
                          for jp in range(NJ // JB):
                              st = {h: st_pool.tile([128, JB * STRIP], F32,
                                                    tag=f"st{h}",
                                                    name=f"st{h}")
                                    for h, _ in heads}
                              # QK^T: pair the two heads' matmuls adjacently so
                              # the 64-row-tiled PE runs them concurrently
                              for u in range(JB):
                                  j = JB * jp + u
                                  kt = kT_s[j // (STRIP // 128)]
                                  ksl = bass.ts(j % (STRIP // 128), 128)
                                  for h, hs in heads:
                                      nc.tensor.matmul(
                                          st[h][:, bass.ts(u, STRIP)],
                                          kt[hs, ksl], qT[hs, :],
                                          start=True, stop=True,
                                      )
                              et = {}
                              for h, _ in heads:
                                  et[h] = et_pool.tile([128, JB * STRIP], BF16,
                                                       tag=f"et{h}",
                                                       name=f"et{h}")
                                  nc.scalar.activation(
                                      et[h][:], st[h][:],
                                      mybir.ActivationFunctionType.Exp,
                                      scale=SCORE_SCALE,
                                  )
                              for h, _ in heads:
                                  for u in range(JB):
                                      j = JB * jp + u
                                      nc.tensor.matmul(
                                          av[h][:], vsb[:, j, h, :],
                                          et[h][:, bass.ts(u, STRIP)],
                                          start=(j == 0), stop=(j == NJ - 1),
                                          skip_group_check=True,
                                      )
                          # normalize: outT_h = av[0:64] * (1/av[64]) broadcast
                          outT = {}
                          for h, _ in heads:
                              outT[h] = outT_pool.tile([D, STRIP], F16,
                                                       tag=f"outT{h}",
                                                       name=f"outT{h}")
                              if USE_GPSIMD_BC:
                                  rec = small.tile([1, STRIP], F32, tag="rec")
                                  nc.vector.reciprocal(rec[:], av[h][64:65, :])
                                  bcb = small.tile([D, STRIP], F32, tag="bcb")
                                  nc.gpsimd.partition_broadcast(bcb[:], rec[:])
                                  nc.vector.tensor_mul(outT[h][:],
                                                       av[h][0:64, :], bcb[:])
                              else:
                                  stage = small.tile([65, STRIP], F32,
                                                     tag="stage")
                                  nc.vector.tensor_copy(stage[:], av[h][:])
                                  rec = small.tile([1, STRIP], F16, tag="rec")
                                  nc.vector.reciprocal(rec[:], stage[64:65, :])
                                  bc = sc_pool.tile([D, STRIP], F32, tag="sc")
                                  nc.tensor.matmul(bc[:], ones_t[:],
                                                   rec[:], start=True, stop=True)
                                  nc.vector.tensor_mul(outT[h][:],
                                                       stage[0:64, :], bc[:])
                          # output projection for this strip's 4 row-tiles
                          for t in range(STRIP // 128):
                              it = i * (STRIP // 128) + t
                              tsl = bass.ts(t, 128)
                              for oc in range(C // STRIP):
                                  osl = bass.ts(oc, STRIP)
                                  pp = sc_pool.tile([128, STRIP], F32, tag="sc")
                                  nc.tensor.matmul(pp[:], outT[0][:, tsl],
                                                   wpa_sb[:, osl],
                                                   start=True, stop=False)
                                  nc.tensor.matmul(pp[:], outT[1][:, tsl],
                                                   wpb_sb[:, osl],
                                                   start=False, stop=True)
                                  ob = ob_pool.tile([128, STRIP], F16, tag="ob")
                                  nc.vector.tensor_copy(ob[:], pp[:])
                                  nc.sync.dma_start(out_r[it][:, osl], ob[:])
    nc.finalize()
    return nc


def _colk(h):
    base = h * D if h < 8 else 2 * 512 + (h - 8) * D
    return slice(base, base + D)


def _colv(h):
    base = 512 + h * D if h < 8 else 3 * 512 + (h - 8) * D
    return slice(base, base + D)


def make_in_maps(x, Wq, Wkv, Wproj):
    x = np.asarray(x, np.float32).reshape(N, C)
    Wq = np.asarray(Wq, np.float32)
    Wkv = np.asarray(Wkv, np.float32)
    Wproj = np.asarray(Wproj, np.float32)
    xT = np.ascontiguousarray(x.T).astype(np.float16)
    in_maps = []
    for core in range(NCORES):
        h0, h1 = 2 * core, 2 * core + 1
        in_maps.append({
            "xT": xT,
            "wq": np.ascontiguousarray(
                np.concatenate([Wq[:, h0 * D:(h0 + 1) * D],
                                Wq[:, h1 * D:(h1 + 1) * D]],
                               axis=1)).astype(np.float16),
            "wk": np.ascontiguousarray(
                np.concatenate([Wkv[:, _colk(h0)], Wkv[:, _colk(h1)]],
                               axis=1)).astype(np.float16),
            "wv": np.ascontiguousarray(
                np.concatenate([Wkv[:, _colv(h0)], Wkv[:, _colv(h1)]],
                               axis=1)).astype(np.float16),
            "wpa": np.ascontiguousarray(
                Wproj[h0 * D:(h0 + 1) * D, :]).astype(np.float16),
            "wpb": np.ascontiguousarray(
                Wproj[h1 * D:(h1 + 1) * D, :]).astype(np.float16),
        })
    return in_maps


_NC = None


def _get_nc():
    global _NC
    if _NC is None:
        _NC = build_nc()
    return _NC


def run_spmd(in_maps, **kwargs):
    return run_bass_kernel_spmd(_get_nc(), in_maps, list(range(NCORES)), **kwargs)


def kernel(x, Wq, Wkv, Wproj, bproj, H=None, W=None, **_unused):
    in_maps = make_in_maps(x, Wq, Wkv, Wproj)
    res = run_spmd(in_maps)
    acc = np.zeros((N, C), np.float64)
    for r in res.results:
        acc += r["out"].astype(np.float64)
    out = acc.astype(np.float32) + np.asarray(bproj, np.float32)[None, :]
    return out.reshape(1, N, C)


if __name__ == "__main__":
    nc = build_nc()
    print("built ok")


# revision 11
# speedup vs baseline: 1.6298x; 1.6298x over previous
"""Trainium2 Bass kernel for DynamicTokenMixing (16-head attention, N=4096, C=1024).

Sharding: head-parallel across 8 NeuronCores, 2 heads per core. Each core
computes q/k/v projections for its 2 heads, full attention for those heads,
and a partial output projection (row-parallel over Wproj); the host sums the
8 partials and adds the bias.

Per-core dataflow (ACT-engine-bound design; exp of 33.5M scores is the floor):
  fp16 inputs (x, weights) -> kT/qT [dims, tokens] fp16, v [tokens, dims] fp16
  ST[j,i] = sum_d k[j,d] q[i,d]      per-head, K=64 row-tiled so the two
                                     heads' matmuls run concurrently (T0/T8)
  ET      = exp(0.5*ST)              bf16 (fp32-like range, no overflow)
  AV^T    = sum_j v_ext[j,:]^T ET[j,:]  (row 64 = softmax denominator)
  outT    = AV^T[0:64] * recip(denom)   recip broadcast via GPSIMD
  out     = sum_h outT_h.T @ Wproj_rows_h  (fp16 partial; host sums 8 cores)
"""

import numpy as np

import concourse.bass as bass
import concourse.mybir as mybir
import concourse.tile as tile
from concourse import bacc
from concourse import library_config
from concourse.bass_utils import run_bass_kernel_spmd

F32 = mybir.dt.float32
F16 = mybir.dt.float16
BF16 = mybir.dt.bfloat16

N = 4096          # tokens
C = 1024          # model dim
D = 64            # head dim
NHEADS = 16
GPD = 2
NCORES = 8
NJ = N // 128     # 32 key tiles
NCT = C // 128    # 8 contraction tiles
STRIP = 512       # query-strip width
NSTRIP = N // STRIP
JB = 2            # key tiles batched per exp instruction
SCORE_SCALE = GPD * GPD * (D ** -0.5)  # 0.5
USE_GPSIMD_BC = True  # broadcast 1/l via GPSIMD; else stage+matmul fallback


def build_nc(repeat=1, hw_loop=False):
    nc = bacc.Bacc("TRN2", target_bir_lowering=False, debug=False,
                   num_devices=NCORES)
    xT = nc.declare_dram_parameter("xT", [C, N], F16, isOutput=False)
    wq = nc.declare_dram_parameter("wq", [C, 128], F16, isOutput=False)
    wk = nc.declare_dram_parameter("wk", [C, 128], F16, isOutput=False)
    wv = nc.declare_dram_parameter("wv", [C, 128], F16, isOutput=False)
    wpa = nc.declare_dram_parameter("wpa", [D, C], F16, isOutput=False)
    wpb = nc.declare_dram_parameter("wpb", [D, C], F16, isOutput=False)
    out = nc.declare_dram_parameter("out", [N, C], F16, isOutput=True)

    xT_r = xT[:].rearrange("(t p) n -> p t n", p=128)    # [128, 8, 4096]
    out_r = out[:].rearrange("(t p) o -> t p o", p=128)  # [32, 128, 1024]

    with tile.TileContext(nc) as tc:
        with (
            nc.allow_low_precision(reason="fp16 matmul inputs by design"),
            tc.tile_pool(name="persist", bufs=1) as persist,
            tc.tile_pool(name="small", bufs=2) as small,
        ):
            wq_sb = persist.tile([128, NCT, 128], F16)
            wk_sb = persist.tile([128, NCT, 128], F16)
            wv_sb = persist.tile([128, NCT, 128], F16)
            wpa_sb = persist.tile([D, C], F16)
            wpb_sb = persist.tile([D, C], F16)
            xt = persist.tile([128, NCT, N], F16)         # whole x^T resident
            kT_s = [persist.tile([128, STRIP], F16, name=f"kT{i}")
                    for i in range(NSTRIP)]
            # v in natural [token, (j, head, d|ones)] layout, one tile so the
            # ones columns are a single memset
            vsb = persist.tile([128, NJ, 2, 65], BF16)

            nc.sync.dma_start(wq_sb[:], wq[:].rearrange("(t p) m -> p t m", p=128))
            nc.sync.dma_start(wk_sb[:], wk[:].rearrange("(t p) m -> p t m", p=128))
            nc.sync.dma_start(wv_sb[:], wv[:].rearrange("(t p) m -> p t m", p=128))
            nc.sync.dma_start(wpa_sb[:], wpa[:])
            nc.sync.dma_start(wpb_sb[:], wpb[:])
            if USE_GPSIMD_BC:
                nc.gpsimd.load_library(library_config.attn)
            nc.gpsimd.memset(vsb[:, :, :, 64:65], 1.0)

            import contextlib
            rep_iter = ([None] if hw_loop and repeat > 1 else range(repeat))
            for _rep in rep_iter:
              with (tc.For_i(0, repeat, 1) if hw_loop and repeat > 1
                    else contextlib.nullcontext()):
                  # ---- Phase 1: k/v projections (q is JIT per-strip later) ----
                  with (
                      tc.tile_pool(name="kv_ps", bufs=2, space="PSUM") as kv_ps,
                  ):
                      for i in range(NSTRIP):
                          sl = bass.ts(i, STRIP)
                          nc.sync.dma_start(xt[:, :, sl], xT_r[:, :, sl])
                          k_ps = kv_ps.tile([128, STRIP], F32, tag="k")
                          for c in range(NCT):
                              nc.tensor.matmul(k_ps[:], wk_sb[:, c, :],
                                               xt[:, c, sl],
                                               start=(c == 0), stop=(c == NCT - 1))
                          nc.vector.tensor_copy(kT_s[i][:], k_ps[:])
                          for t in range(STRIP // 128):
                              j = i * (STRIP // 128) + t
                              tsl = slice(i * STRIP + t * 128,
                                          i * STRIP + (t + 1) * 128)
                              v_ps = kv_ps.tile([128, 2, 64], F32, tag="v")
                              for c in range(NCT):
                                  nc.tensor.matmul(v_ps[:], xt[:, c, tsl],
                                                   wv_sb[:, c, :],
                                                   start=(c == 0),
                                                   stop=(c == NCT - 1))
                              nc.vector.tensor_copy(vsb[:, j, :, 0:64], v_ps[:])

                  # ---- Phase 2: attention + projection, pipelined per strip ----
                  with (
                      tc.tile_pool(name="qT", bufs=2) as qT_pool,
                      tc.tile_pool(name="att_et", bufs=3) as et_pool,
                      tc.tile_pool(name="outT", bufs=2) as outT_pool,
                      tc.tile_pool(name="ob", bufs=3) as ob_pool,
                      tc.tile_pool(name="att_st", bufs=1, space="PSUM") as st_pool,
                      tc.tile_pool(name="att_av", bufs=1, space="PSUM") as av_pool,
                      tc.tile_pool(name="qp_ps", bufs=1, space="PSUM") as qp_pool,
                      tc.tile_pool(name="pr_ps", bufs=1, space="PSUM") as pr_pool,
                  ):
                      for i in range(NSTRIP):
                          sl = bass.ts(i, STRIP)
                          # JIT q projection for this strip
                          q_ps = qp_pool.tile([128, STRIP], F32, tag="q")
                          for c in range(NCT):
                              nc.tensor.matmul(q_ps[:], wq_sb[:, c, :],
                                               xt[:, c, sl],
                                               start=(c == 0), stop=(c == NCT - 1))
                          qT = qT_pool.tile([128, STRIP], F16, tag="qT")
                          nc.vector.tensor_copy(qT[:], q_ps[:])

                          av = {h: av_pool.tile([65, STRIP], F32, tag=f"av{h}",
                                                name=f"av{h}")
                                for h in (0, 1)}
                          heads = ((0, slice(0, 64)), (1, slice(64, 128)))

                          def emit_qk(jp, h, hs, st_tile):
                              for u in range(JB):
                                  j = JB * jp + u
                                  kt = kT_s[j // (STRIP // 128)]
                                  ksl = bass.ts(j % (STRIP // 128), 128)
                                  nc.tensor.matmul(
                                      st_tile[:, bass.ts(u, STRIP)],
                                      kt[hs, ksl], qT[hs, :],
                                      start=True, stop=True,
                                  )

                          # software-pipelined: QK for jp+1 issues right after
                          # exp(jp) frees the st buffer, before AV(jp), so the
                          # ACT engine never waits on the PE queue
                          st = {}
                          for h, hs in heads:
                              st[h] = st_pool.tile([128, JB * STRIP], F32,
                                                   tag=f"st{h}", name=f"st{h}")
                              emit_qk(0, h, hs, st[h])
                          for jp in range(NJ // JB):
                              for h, hs in heads:
                                  et = et_pool.tile([128, JB * STRIP], BF16,
                                                    tag=f"et{h}", name=f"et{h}")
                                  nc.scalar.activation(
                                      et[:], st[h][:],
                                      mybir.ActivationFunctionType.Exp,
                                      scale=SCORE_SCALE,
                                  )
                                  if jp + 1 < NJ // JB:
                                      st[h] = st_pool.tile([128, JB * STRIP],
                                                           F32, tag=f"st{h}",
                                                           name=f"st{h}")
                                      emit_qk(jp + 1, h, hs, st[h])
                                  for u in range(JB):
                                      j = JB * jp + u
                                      nc.tensor.matmul(
                                          av[h][:], vsb[:, j, h, :],
                                          et[:, bass.ts(u, STRIP)],
                                          start=(j == 0), stop=(j == NJ - 1),
                                          skip_group_check=True,
                                      )
                          # normalize: outT_h = av[0:64] * (1/av[64]);
                          # stage-copy frees the av PSUM bank immediately, the
                          # GPSIMD broadcast runs off the critical path
                          outT = {}
                          for h, _ in heads:
                              outT[h] = outT_pool.tile([D, STRIP], F16,
                                                       tag=f"outT{h}",
                                                       name=f"outT{h}")
                              stage = small.tile([65, STRIP], F32, tag="stage")
                              nc.vector.tensor_copy(stage[:], av[h][:])
                              rec = small.tile([1, STRIP], F32, tag="rec")
                              nc.vector.reciprocal(rec[:], stage[64:65, :])
                              bcb = small.tile([D, STRIP], F32, tag="bcb")
                              nc.gpsimd.partition_broadcast(bcb[:], rec[:])
                              nc.vector.tensor_mul(outT[h][:],
                                                   stage[0:64, :], bcb[:])
                          # output projection for this strip's 4 row-tiles
                          for t in range(STRIP // 128):
                              it = i * (STRIP // 128) + t
                              tsl = bass.ts(t, 128)
                              for oc in range(C // STRIP):
                                  osl = bass.ts(oc, STRIP)
                                  pp = pr_pool.tile([128, STRIP], F32, tag="pp")
                                  nc.tensor.matmul(pp[:], outT[0][:, tsl],
                                                   wpa_sb[:, osl],
                                                   start=True, stop=False)
                                  nc.tensor.matmul(pp[:], outT[1][:, tsl],
                                                   wpb_sb[:, osl],
                                                   start=False, stop=True)
                                  ob = ob_pool.tile([128, STRIP], F16, tag="ob")
                                  nc.vector.tensor_copy(ob[:], pp[:])
                                  nc.sync.dma_start(out_r[it][:, osl], ob[:])
    nc.finalize()
    return nc


def _colk(h):
    base = h * D if h < 8 else 2 * 512 + (h - 8) * D
    return slice(base, base + D)


def _colv(h):
    base = 512 + h * D if h < 8 else 3 * 512 + (h - 8) * D
    return slice(base, base + D)


def make_in_maps(x, Wq, Wkv, Wproj):
    x = np.asarray(x, np.float32).reshape(N, C)
    Wq = np.asarray(Wq, np.float32)
    Wkv = np.asarray(Wkv, np.float32)
    Wproj = np.asarray(Wproj, np.float32)
    xT = np.ascontiguousarray(x.T).astype(np.float16)
    in_maps = []
    for core in range(NCORES):
        h0, h1 = 2 * core, 2 * core + 1
        in_maps.append({
            "xT": xT,
            "wq": np.ascontiguousarray(
                np.concatenate([Wq[:, h0 * D:(h0 + 1) * D],
                                Wq[:, h1 * D:(h1 + 1) * D]],
                               axis=1)).astype(np.float16),
            "wk": np.ascontiguousarray(
                np.concatenate([Wkv[:, _colk(h0)], Wkv[:, _colk(h1)]],
                               axis=1)).astype(np.float16),
            "wv": np.ascontiguousarray(
                np.concatenate([Wkv[:, _colv(h0)], Wkv[:, _colv(h1)]],
                               axis=1)).astype(np.float16),
            "wpa": np.ascontiguousarray(
                Wproj[h0 * D:(h0 + 1) * D, :]).astype(np.float16),
            "wpb": np.ascontiguousarray(
                Wproj[h1 * D:(h1 + 1) * D, :]).astype(np.float16),
        })
    return in_maps


_NC = None


def _get_nc():
    global _NC
    if _NC is None:
        _NC = build_nc()
    return _NC


def run_spmd(in_maps, **kwargs):
    return run_bass_kernel_spmd(_get_nc(), in_maps, list(range(NCORES)), **kwargs)


def kernel(x, Wq, Wkv, Wproj, bproj, H=None, W=None, **_unused):
    in_maps = make_in_maps(x, Wq, Wkv, Wproj)
    res = run_spmd(in_maps)
    acc = np.zeros((N, C), np.float64)
    for r in res.results:
        acc += r["out"].astype(np.float64)
    out = acc.astype(np.float32) + np.asarray(bproj, np.float32)[None, :]
    return out.reshape(1, N, C)


if __name__ == "__main__":
    nc = build_nc()
    print("built ok")


# revision 12
# speedup vs baseline: 1.8263x; 1.1206x over previous
"""Trainium2 Bass kernel for DynamicTokenMixing (16-head attention, N=4096, C=1024).

Sharding: head-parallel across 8 NeuronCores, 2 heads per core. Each core
computes q/k/v projections for its 2 heads, full attention for those heads,
and a partial output projection (row-parallel over Wproj); the host sums the
8 partials and adds the bias.

Per-core dataflow (ACT-engine-bound design; exp of 33.5M scores is the floor):
  fp16 inputs (x, weights) -> kT/qT [dims, tokens] fp16, v [tokens, dims] fp16
  ST[j,i] = sum_d k[j,d] q[i,d]      per-head, K=64 row-tiled so the two
                                     heads' matmuls run concurrently (T0/T8)
  ET      = exp(0.5*ST)              bf16 (fp32-like range, no overflow)
  AV^T    = sum_j v_ext[j,:]^T ET[j,:]  (row 64 = softmax denominator)
  outT    = AV^T[0:64] * recip(denom)   recip broadcast via GPSIMD
  out     = sum_h outT_h.T @ Wproj_rows_h  (fp16 partial; host sums 8 cores)
"""

import numpy as np

import concourse.bass as bass
import concourse.mybir as mybir
import concourse.tile as tile
from concourse import bacc
from concourse import library_config
from concourse.bass_utils import run_bass_kernel_spmd

F32 = mybir.dt.float32
F16 = mybir.dt.float16
BF16 = mybir.dt.bfloat16

N = 4096          # tokens
C = 1024          # model dim
D = 64            # head dim
NHEADS = 16
GPD = 2
NCORES = 8
NJ = N // 128     # 32 key tiles
NCT = C // 128    # 8 contraction tiles
STRIP = 512       # query-strip width
NSTRIP = N // STRIP
JB = 2            # key tiles batched per exp instruction
SCORE_SCALE = GPD * GPD * (D ** -0.5)  # 0.5
USE_GPSIMD_BC = True  # broadcast 1/l via GPSIMD; else stage+matmul fallback


def build_nc(repeat=1, hw_loop=False):
    nc = bacc.Bacc("TRN2", target_bir_lowering=False, debug=False,
                   num_devices=NCORES)
    xT = nc.declare_dram_parameter("xT", [C, N], F16, isOutput=False)
    wq = nc.declare_dram_parameter("wq", [C, 128], F16, isOutput=False)
    wk = nc.declare_dram_parameter("wk", [C, 128], F16, isOutput=False)
    wv = nc.declare_dram_parameter("wv", [C, 128], F16, isOutput=False)
    wpa = nc.declare_dram_parameter("wpa", [D, C], F16, isOutput=False)
    wpb = nc.declare_dram_parameter("wpb", [D, C], F16, isOutput=False)
    out = nc.declare_dram_parameter("out", [N, C], F16, isOutput=True)

    xT_r = xT[:].rearrange("(t p) n -> p t n", p=128)    # [128, 8, 4096]
    out_r = out[:].rearrange("(t p) o -> t p o", p=128)  # [32, 128, 1024]

    with tile.TileContext(nc) as tc:
        with (
            nc.allow_low_precision(reason="fp16 matmul inputs by design"),
            tc.tile_pool(name="persist", bufs=1) as persist,
            tc.tile_pool(name="small", bufs=2) as small,
        ):
            wq_sb = persist.tile([128, NCT, 128], F16)
            wk_sb = persist.tile([128, NCT, 128], F16)
            wv_sb = persist.tile([128, NCT, 128], F16)
            wpa_sb = persist.tile([D, C], F16)
            wpb_sb = persist.tile([D, C], F16)
            xt = persist.tile([128, NCT, N], F16)         # whole x^T resident
            # k for each head in a zero-padded [128, n] layout: the other
            # head's rows stay zero so QK^T runs with K=128 (full PE mode,
            # no 64-row tiling-mode switches between QK and AV matmuls)
            kz = [persist.tile([128, NSTRIP, STRIP], F16, name=f"kz{h}")
                  for h in (0, 1)]
            # v in natural [token, (j, head, d|ones)] layout, one tile so the
            # ones columns are a single memset
            vsb = persist.tile([128, NJ, 2, 65], BF16)

            nc.sync.dma_start(wq_sb[:], wq[:].rearrange("(t p) m -> p t m", p=128))
            nc.sync.dma_start(wk_sb[:], wk[:].rearrange("(t p) m -> p t m", p=128))
            nc.sync.dma_start(wv_sb[:], wv[:].rearrange("(t p) m -> p t m", p=128))
            nc.sync.dma_start(wpa_sb[:], wpa[:])
            nc.sync.dma_start(wpb_sb[:], wpb[:])
            if USE_GPSIMD_BC:
                nc.gpsimd.load_library(library_config.attn)
            nc.gpsimd.memset(vsb[:, :, :, 64:65], 1.0)
            nc.gpsimd.memset(kz[0][64:128, :, :], 0.0)
            nc.gpsimd.memset(kz[1][0:64, :, :], 0.0)

            import contextlib
            rep_iter = ([None] if hw_loop and repeat > 1 else range(repeat))
            for _rep in rep_iter:
              with (tc.For_i(0, repeat, 1) if hw_loop and repeat > 1
                    else contextlib.nullcontext()):
                  # ---- Phase 1: k/v projections (q is JIT per-strip later) ----
                  with (
                      tc.tile_pool(name="kv_ps", bufs=2, space="PSUM") as kv_ps,
                  ):
                      for i in range(NSTRIP):
                          sl = bass.ts(i, STRIP)
                          nc.sync.dma_start(xt[:, :, sl], xT_r[:, :, sl])
                          k_ps = kv_ps.tile([128, STRIP], F32, tag="k")
                          for c in range(NCT):
                              nc.tensor.matmul(k_ps[:], wk_sb[:, c, :],
                                               xt[:, c, sl],
                                               start=(c == 0), stop=(c == NCT - 1))
                          nc.vector.tensor_copy(kz[0][0:64, i, :],
                                                k_ps[0:64, :])
                          nc.vector.tensor_copy(kz[1][64:128, i, :],
                                                k_ps[64:128, :])
                          for t in range(STRIP // 128):
                              j = i * (STRIP // 128) + t
                              tsl = slice(i * STRIP + t * 128,
                                          i * STRIP + (t + 1) * 128)
                              v_ps = kv_ps.tile([128, 2, 64], F32, tag="v")
                              for c in range(NCT):
                                  nc.tensor.matmul(v_ps[:], xt[:, c, tsl],
                                                   wv_sb[:, c, :],
                                                   start=(c == 0),
                                                   stop=(c == NCT - 1))
                              nc.vector.tensor_copy(vsb[:, j, :, 0:64], v_ps[:])

                  # ---- Phase 2: attention + projection, pipelined per strip ----
                  with (
                      tc.tile_pool(name="qT", bufs=2) as qT_pool,
                      tc.tile_pool(name="att_et", bufs=3) as et_pool,
                      tc.tile_pool(name="outT", bufs=2) as outT_pool,
                      tc.tile_pool(name="ob", bufs=3) as ob_pool,
                      tc.tile_pool(name="att_st", bufs=1, space="PSUM") as st_pool,
                      tc.tile_pool(name="att_av", bufs=1, space="PSUM") as av_pool,
                      tc.tile_pool(name="qp_ps", bufs=1, space="PSUM") as qp_pool,
                      tc.tile_pool(name="pr_ps", bufs=1, space="PSUM") as pr_pool,
                  ):
                      for i in range(NSTRIP):
                          sl = bass.ts(i, STRIP)
                          # JIT q projection for this strip
                          q_ps = qp_pool.tile([128, STRIP], F32, tag="q")
                          for c in range(NCT):
                              nc.tensor.matmul(q_ps[:], wq_sb[:, c, :],
                                               xt[:, c, sl],
                                               start=(c == 0), stop=(c == NCT - 1))
                          qT = qT_pool.tile([128, STRIP], F16, tag="qT")
                          nc.vector.tensor_copy(qT[:], q_ps[:])

                          av = {h: av_pool.tile([65, STRIP], F32, tag=f"av{h}",
                                                name=f"av{h}")
                                for h in (0, 1)}
                          heads = ((0, slice(0, 64)), (1, slice(64, 128)))

                          def emit_qk(jp, h, hs, st_tile):
                              for u in range(JB):
                                  j = JB * jp + u
                                  ks = j // (STRIP // 128)
                                  ksl = bass.ts(j % (STRIP // 128), 128)
                                  nc.tensor.matmul(
                                      st_tile[:, bass.ts(u, STRIP)],
                                      kz[h][:, ks, ksl], qT[:],
                                      start=True, stop=True,
                                  )

                          # software-pipelined: QK for jp+1 issues right after
                          # exp(jp) frees the st buffer, before AV(jp), so the
                          # ACT engine never waits on the PE queue
                          st = {}
                          for h, hs in heads:
                              st[h] = st_pool.tile([128, JB * STRIP], F32,
                                                   tag=f"st{h}", name=f"st{h}")
                              emit_qk(0, h, hs, st[h])
                          for jp in range(NJ // JB):
                              for h, hs in heads:
                                  et = et_pool.tile([128, JB * STRIP], BF16,
                                                    tag=f"et{h}", name=f"et{h}")
                                  nc.scalar.activation(
                                      et[:], st[h][:],
                                      mybir.ActivationFunctionType.Exp,
                                      scale=SCORE_SCALE,
                                  )
                                  if jp + 1 < NJ // JB:
                                      st[h] = st_pool.tile([128, JB * STRIP],
                                                           F32, tag=f"st{h}",
                                                           name=f"st{h}")
                                      emit_qk(jp + 1, h, hs, st[h])
                                  for u in range(JB):
                                      j = JB * jp + u
                                      nc.tensor.matmul(
                                          av[h][:], vsb[:, j, h, :],
                                          et[:, bass.ts(u, STRIP)],
                                          start=(j == 0), stop=(j == NJ - 1),
                                          skip_group_check=True,
                                      )
                          # normalize: outT_h = av[0:64] * (1/av[64]);
                          # stage-copy frees the av PSUM bank immediately, the
                          # GPSIMD broadcast runs off the critical path
                          outT = {}
                          for h, _ in heads:
                              outT[h] = outT_pool.tile([D, STRIP], F16,
                                                       tag=f"outT{h}",
                                                       name=f"outT{h}")
                              stage = small.tile([65, STRIP], F32, tag="stage")
                              nc.vector.tensor_copy(stage[:], av[h][:])
                              rec = small.tile([1, STRIP], F32, tag="rec")
                              nc.vector.reciprocal(rec[:], stage[64:65, :])
                              bcb = small.tile([D, STRIP], F32, tag="bcb")
                              nc.gpsimd.partition_broadcast(bcb[:], rec[:])
                              nc.vector.tensor_mul(outT[h][:],
                                                   stage[0:64, :], bcb[:])
                          # output projection for this strip's 4 row-tiles
                          for t in range(STRIP // 128):
                              it = i * (STRIP // 128) + t
                              tsl = bass.ts(t, 128)
                              for oc in range(C // STRIP):
                                  osl = bass.ts(oc, STRIP)
                                  pp = pr_pool.tile([128, STRIP], F32, tag="pp")
                                  nc.tensor.matmul(pp[:], outT[0][:, tsl],
                                                   wpa_sb[:, osl],
                                                   start=True, stop=False)
                                  nc.tensor.matmul(pp[:], outT[1][:, tsl],
                                                   wpb_sb[:, osl],
                                                   start=False, stop=True)
                                  ob = ob_pool.tile([128, STRIP], F16, tag="ob")
                                  nc.vector.tensor_copy(ob[:], pp[:])
                                  nc.sync.dma_start(out_r[it][:, osl], ob[:])
    nc.finalize()
    return nc


def _colk(h):
    base = h * D if h < 8 else 2 * 512 + (h - 8) * D
    return slice(base, base + D)


def _colv(h):
    base = 512 + h * D if h < 8 else 3 * 512 + (h - 8) * D
    return slice(base, base + D)


def make_in_maps(x, Wq, Wkv, Wproj):
    x = np.asarray(x, np.float32).reshape(N, C)
    Wq = np.asarray(Wq, np.float32)
    Wkv = np.asarray(Wkv, np.float32)
    Wproj = np.asarray(Wproj, np.float32)
    xT = np.ascontiguousarray(x.T).astype(np.float16)
    in_maps = []
    for core in range(NCORES):
        h0, h1 = 2 * core, 2 * core + 1
        in_maps.append({
            "xT": xT,
            "wq": np.ascontiguousarray(
                np.concatenate([Wq[:, h0 * D:(h0 + 1) * D],
                                Wq[:, h1 * D:(h1 + 1) * D]],
                               axis=1)).astype(np.float16),
            "wk": np.ascontiguousarray(
                np.concatenate([Wkv[:, _colk(h0)], Wkv[:, _colk(h1)]],
                               axis=1)).astype(np.float16),
            "wv": np.ascontiguousarray(
                np.concatenate([Wkv[:, _colv(h0)], Wkv[:, _colv(h1)]],
                               axis=1)).astype(np.float16),
            "wpa": np.ascontiguousarray(
                Wproj[h0 * D:(h0 + 1) * D, :]).astype(np.float16),
            "wpb": np.ascontiguousarray(
                Wproj[h1 * D:(h1 + 1) * D, :]).astype(np.float16),
        })
    return in_maps


_NC = None


def _get_nc():
    global _NC
    if _NC is None:
        _NC = build_nc()
    return _NC


def run_spmd(in_maps, **kwargs):
    return run_bass_kernel_spmd(_get_nc(), in_maps, list(range(NCORES)), **kwargs)


def kernel(x, Wq, Wkv, Wproj, bproj, H=None, W=None, **_unused):
    in_maps = make_in_maps(x, Wq, Wkv, Wproj)
    res = run_spmd(in_maps)
    acc = np.zeros((N, C), np.float64)
    for r in res.results:
        acc += r["out"].astype(np.float64)
    out = acc.astype(np.float32) + np.asarray(bproj, np.float32)[None, :]
    return out.reshape(1, N, C)


if __name__ == "__main__":
    nc = build_nc()
    print("built ok")
